# revision 38
# baseline (speedup 1.0000x reference)
"""Multi-head attention (B=4, S=2048, D=768, H=12) on 8 Trainium2 cores.

Sharding: the 48 (batch, head) pairs are data-parallel; each core gets 6.

Math restructure (exact):
  scores = (XWq^T+bq)(XWk^T+bk)^T -> softmax-invariant terms dropped:
    s_ij = x_i Wqk x_j^T + c_j   with Wqk = Wq^T Wk, c = X (bq Wk)^T
  The per-k-column bias c_j rides into the exp for free (per-partition
  bias port on ACT, add-constant on DVE), so K^T is the raw input and
  only one projection (qhat = X Wqk) is computed on device.
  V bias folds into a rank-1 (K=1) accumulating matmul.
  Normalization happens on the HOST: the kernel ships numerator rows
  plus a denominator row ([65, S] per head) produced by an appended
  ones-column in V_aug.

Engine split (the softmax exp is the throughput wall):
  per score pair, tile A -> ACT native exp (PSUM->SBUF bf16),
  tile B -> DVE Schraudolph: i16 = s*(A/8) + (B + A*c/8), bitcast bf16.
  (max|s/8| ~ 11, so i16 stays in [14k, 19k]: no overflow, no sign flip.)

Precision: fp16 x/qhat for the score matmuls (cuts bf16 rounding 8x),
bf16 P and V_aug, fp32 PSUM everywhere; host-measured rel err ~9.6e-3
at a 50/50 ACT/DVE split (gate is 2e-2).

PSUM: mm pool 3x[128,1024]f32 (score pairs / qhat / V) + av pool
2x[65,512]f32 = exactly 8 banks.
"""

import sys
from collections import deque

for _p in ("/opt/trn_rl_repo",):
    if _p not in sys.path:
        sys.path.insert(0, _p)

import numpy as np

B, S, D, H = 4, 2048, 768, 12
DH = 64
NCORES = 8
HPC = (B * H) // NCORES  # 6 heads per core
NKC = S // 128  # 16 k-chunks
NQB = 2  # q blocks of 1024
QB = S // NQB
PUMPS_PER_PAIR = 2

TRICK_A = 128.0 / np.log(2.0)  # 184.664965
TRICK_B = 127.0 * 128.0 - 5.57  # bf16 exponent bias, sawtooth-centered


def _split_multi_waits(nc):
    """This walrus build rejects >1 sync wait per instruction. Insert
    single-wait NoOps (same engine, so same instruction stream) ahead of
    any instruction carrying several waits."""
    import bass_rust
    import concourse.mybir as mybir

    n_split = 0
    for f in nc.m.functions:
        for bb in f.blocks:
            out = []
            dirty = False
            for inst in bb.instructions:
                si = inst.sync_info
                if si is not None and len(si.on_wait) > 1:
                    waits = list(si.on_wait)
                    for j, w in enumerate(waits[:-1]):
                        nop = mybir.InstNoOp(name=f"{inst.name}-w{j}", ins=[], outs=[])
                        nop.engine = inst.engine
                        nop.sync_info = bass_rust.SyncInfo(on_wait=[w], on_update=[])
                        out.append(nop)
                    si.on_wait = waits[-1:]
                    dirty = True
                    n_split += 1
                out.append(inst)
            if dirty:
                bb.instructions = out
    return n_split


_BUILT = None


def build():
    global _BUILT
    if _BUILT is not None:
        return _BUILT
    import concourse.bass as bass
    import concourse.mybir as mybir
    import concourse.tile as tile

    F32 = mybir.dt.float32
    F16 = mybir.dt.float16
    BF = mybir.dt.bfloat16
    I16 = mybir.dt.int16
    AF = mybir.ActivationFunctionType
    ALU = mybir.AluOpType

    nc = bass.Bass()
    xTd = nc.dram_tensor("xT", [HPC, 128, S], F16, kind="ExternalInput")
    wqkd = nc.dram_tensor("wqk", [HPC, 64, 64], F16, kind="ExternalInput")
    wvTd = nc.dram_tensor("wvT", [HPC, 128, 64], F16, kind="ExternalInput")
    cbd = nc.dram_tensor("cb", [HPC, 128, NKC], F32, kind="ExternalInput")
    tbd = nc.dram_tensor("tb", [HPC, 128, NKC], F32, kind="ExternalInput")
    outd = nc.dram_tensor("out", [HPC, 65, S], F32, kind="ExternalOutput")

    with tile.TileContext(nc) as tc:
        with (
            tc.tile_pool(name="x", bufs=2) as xpool,
            tc.tile_pool(name="w", bufs=2) as wpool,
            tc.tile_pool(name="qh", bufs=2) as qpool,
            tc.tile_pool(name="v", bufs=2) as vpool,
            tc.tile_pool(name="pt", bufs=2 * NKC) as ptpool,
            tc.tile_pool(name="ot", bufs=2) as otpool,
            tc.tile_pool(name="mm", bufs=2, space="PSUM") as mmpool,
            tc.tile_pool(name="av", bufs=4, space="PSUM") as avpool,
        ):
            # warm the ACT exp table during the first DMAs
            warm = xpool.tile([1, 1], F32, tag="warm")
            nc.vector.memset(warm[:], 0.0)
            nc.scalar.activation(warm[:], warm[:], AF.Exp)

            state = {}

            def dma_steps(i):
                x_t = xpool.tile([128, S], F16, tag="x", name=f"x{i}")
                nc.gpsimd.dma_start(x_t[:], xTd[i])
                wqk_t = wpool.tile([64, 64], F16, tag="wqk", name=f"wqk{i}")
                nc.sync.dma_start(wqk_t[:], wqkd[i])
                wv_t = wpool.tile([128, 64], F16, tag="wv", name=f"wv{i}")
                nc.sync.dma_start(wv_t[:], wvTd[i])
                cb_t = wpool.tile([128, NKC], F32, tag="cb", name=f"cb{i}")
                nc.sync.dma_start(cb_t[:], cbd[i])
                tb_t = wpool.tile([128, NKC], F32, tag="tb", name=f"tb{i}")
                nc.sync.dma_start(tb_t[:], tbd[i])
                state.setdefault(i, {})["in"] = (x_t, wqk_t, wv_t, cb_t, tb_t)
                yield

            def qkv_steps(i):
                x_t, wqk_t, wv_t, cb_t, tb_t = state[i]["in"]

                # qhat^T = Wqk^T X^T, duplicated into both partition halves
                qhat = qpool.tile([128, S], F16, tag="qh", name=f"qh{i}")
                for qm in range(NQB):
                    ps = mmpool.tile([128, QB], F32, tag="mm", name=f"qp{i}_{qm}")
                    sl = slice(qm * QB, (qm + 1) * QB)
                    for hh in range(2):
                        q0 = qm * QB + hh * 512
                        rhs = x_t[0:64, q0 : q0 + 512]
                        psl = slice(hh * 512, (hh + 1) * 512)
                        nc.tensor.matmul(
                            ps[0:64, psl], wqk_t[:], rhs, tile_position=(0, 0)
                        )
                        nc.tensor.matmul(
                            ps[64:128, psl], wqk_t[:], rhs, tile_position=(0, 64)
                        )
                    nc.scalar.activation(qhat[:, sl], ps[:], AF.Copy)
                    yield

                # V_aug: [k-in-chunk, chunk, e + ones]; V bias added on host
                vhat = vpool.tile([128, NKC, 65], BF, tag="v", name=f"v{i}")
                nc.vector.memset(vhat[:, :, 64:65], 1.0)
                vps = mmpool.tile([128, NKC * 64], F32, tag="mm", name=f"vp{i}")
                for p in range(8):
                    nc.tensor.matmul(
                        vps[:, p * 64 : (p + 1) * 64],
                        x_t[0:64, p * 128 : (p + 1) * 128],
                        wv_t[0:64, :],
                        tile_position=(0, 0),
                    )
                    nc.tensor.matmul(
                        vps[:, (p + 8) * 64 : (p + 9) * 64],
                        x_t[64:128, (p + 8) * 128 : (p + 9) * 128],
                        wv_t[64:128, :],
                        tile_position=(64, 0),
                    )
                    if p % 2 == 1:
                        yield
                nc.vector.tensor_copy(
                    vhat[:, :, 0:64],
                    vps[:].rearrange("p (c e) -> p c e", e=64),
                )
                yield
                state[i].update(
                    {"x": x_t, "qh": qhat, "v": vhat, "cb": cb_t, "tb": tb_t, "pt": {}}
                )

            def sc_pair(i, jb, p):
                """k-chunk pair (p, p+8) scores + exps for q block jb.
                Chunk p -> ACT native exp; chunk p+8 -> DVE bit-trick."""
                st = state[i]
                x_t, qhat = st["x"], st["qh"]
                pt = st["pt"].setdefault(jb, [None] * NKC)
                qsl = slice(jb * QB, (jb + 1) * QB)
                # interleave A/B halves: starts are pc-monotone, so issuing
                # A0,A1,B0,B1 would stall B0 behind A1's row-group wait
                tA = mmpool.tile([128, QB], F32, tag="mm", name=f"sA{i}_{jb}_{p}")
                tB = mmpool.tile([128, QB], F32, tag="mm", name=f"sB{i}_{jb}_{p}")
                lA = x_t[0:64, p * 128 : (p + 1) * 128]
                lB = x_t[64:128, (p + 8) * 128 : (p + 9) * 128]
                for hh in range(2):
                    q0 = jb * QB + hh * 512
                    sl = slice(hh * 512, (hh + 1) * 512)
                    nc.tensor.matmul(
                        tA[:, sl], lA, qhat[0:64, q0 : q0 + 512], tile_position=(0, 0)
                    )
                    nc.tensor.matmul(
                        tB[:, sl],
                        lB,
                        qhat[64:128, q0 : q0 + 512],
                        tile_position=(64, 0),
                    )
                pA = ptpool.tile([128, QB], BF, tag="pt", name=f"pA{i}_{jb}_{p}")
                nc.scalar.activation(
                    pA[:], tA[:], AF.Exp, bias=st["cb"][:, p : p + 1], scale=0.125
                )
                if jb == 0 and p == 0:
                    # one pair per head exact on ACT: keeps ACT/DVE balanced
                    pB = ptpool.tile([128, QB], BF, tag="pt", name=f"pB{i}_{jb}_{p}")
                    nc.scalar.activation(
                        pB[:],
                        tB[:],
                        AF.Exp,
                        bias=st["cb"][:, p + 8 : p + 9],
                        scale=0.125,
                    )
                    pt[p + 8] = pB[:]
                else:
                    pB = ptpool.tile([128, QB], I16, tag="pt", name=f"pB{i}_{jb}_{p}")
                    nc.vector.tensor_scalar(
                        pB[:],
                        tB[:],
                        TRICK_A / 8.0,
                        st["tb"][:, p + 8 : p + 9],
                        ALU.mult,
                        ALU.add,
                    )
                    pt[p + 8] = pB[:].bitcast(BF)
                pt[p] = pA[:]

            def av_steps(i, jb):
                """AV numerator + denominator row. The K=128 contraction is
                split into two row-group chains (k-rows 0:64 -> bank A at
                tile (0,0), rows 64:128 -> bank B at (64,0)): alternating
                half-row matmuls let each LDWEIGHTS overlap the other
                chain's matmul. The A+B merge is a DVE add that doubles as
                the PSUM->SBUF move. Output leaves unnormalized ([65, S]
                fp32); the host divides."""
                st = state[i]
                vhat = st["v"]
                pt = st["pt"][jb]
                nq = QB // 512
                avA = [
                    avpool.tile([65, 512], F32, tag="av", name=f"avA{i}_{jb}_{qm}")
                    for qm in range(nq)
                ]
                avB = [
                    avpool.tile([65, 512], F32, tag="av", name=f"avB{i}_{jb}_{qm}")
                    for qm in range(nq)
                ]
                for kc in range(NKC):
                    for qm in range(nq):
                        sl = slice(qm * 512, (qm + 1) * 512)
                        nc.tensor.matmul(
                            avA[qm][:],
                            vhat[0:64, kc, :],
                            pt[kc][0:64, sl],
                            tile_position=(0, 0),
                            start=(kc == 0),
                            stop=(kc == NKC - 1),
                        )
                        nc.tensor.matmul(
                            avB[qm][:],
                            vhat[64:128, kc, :],
                            pt[kc][64:128, sl],
                            tile_position=(64, 0),
                            start=(kc == 0),
                            stop=(kc == NKC - 1),
                        )
                    yield
                del st["pt"][jb]
                for qm in range(nq):
                    ots = otpool.tile(
                        [65, 512], F32, tag="ot", name=f"ot{i}_{jb}_{qm}"
                    )
                    nc.scalar.activation(ots[:], avA[qm][:], AF.Copy)
                    nc.vector.tensor_add(ots[:], ots[:], avB[qm][:])
                    nc.sync.dma_start(
                        outd[i][:, jb * QB + qm * 512 : jb * QB + (qm + 1) * 512],
                        ots[:],
                    )
                    yield

            fillers = deque()

            def pump(n):
                while n > 0 and fillers:
                    try:
                        next(fillers[0])
                        n -= 1
                    except StopIteration:
                        fillers.popleft()

            def drain(gen=None):
                while fillers and (gen is None or gen in fillers):
                    pump(1)

            def unit(i, jb):
                for p in range(NKC // 2):
                    sc_pair(i, jb, p)
                    pump(PUMPS_PER_PAIR)

            for _ in dma_steps(0):
                pass
            g0 = qkv_steps(0)
            fillers.append(g0)
            drain(g0)
            if HPC > 1:
                fillers.append(dma_steps(1))
            unit(0, 0)
            for i in range(HPC):
                if i > 0:
                    if i + 1 < HPC:
                        fillers.append(dma_steps(i + 1))
                    fillers.append(av_steps(i - 1, 1))
                    unit(i, 0)
                fillers.append(av_steps(i, 0))
                if i + 1 < HPC:
                    g = qkv_steps(i + 1)
                    fillers.append(g)
                    unit(i, 1)
                    drain(g)
                else:
                    unit(i, 1)
            fillers.append(av_steps(HPC - 1, 1))
            drain()

    _split_multi_waits(nc)
    _BUILT = nc
    return nc


def _core_inputs(sequences, wq, bq, wk, bk, wv, bv):
    f16 = np.float16
    xh = np.asarray(sequences, dtype=np.float32).reshape(B, S, H, DH)
    wq = np.asarray(wq, np.float32)
    bq = np.asarray(bq, np.float32)
    wk = np.asarray(wk, np.float32)
    wv = np.asarray(wv, np.float32)
    bv = np.asarray(bv, np.float32)
    in_maps = []
    for c in range(NCORES):
        xT = np.empty((HPC, 128, S), dtype=f16)
        wqk = np.empty((HPC, 64, 64), dtype=f16)
        wvT = np.empty((HPC, 128, 64), dtype=f16)
        cb = np.empty((HPC, 128, NKC), dtype=np.float32)
        tb = np.empty((HPC, 128, NKC), dtype=np.float32)
        for i in range(HPC):
            f = c * HPC + i
            b, h = f // H, f % H
            xbh = xh[b, :, h, :]  # [S, 64]
            xt = np.ascontiguousarray(xbh.T).astype(f16)
            xT[i, 0:64] = xt
            xT[i, 64:128] = xt
            wqk[i] = (wq[h].T @ wk[h]).astype(f16)
            wvT[i, 0:64] = wv[h].T.astype(f16)
            wvT[i, 64:128] = wv[h].T.astype(f16)
            btil = bq[h] @ wk[h]  # [64]
            c8 = (xbh @ btil) / 8.0  # [S]
            cb[i] = c8.reshape(NKC, 128).T
            tb[i] = TRICK_B + TRICK_A * cb[i]
        in_maps.append({"xT": xT, "wqk": wqk, "wvT": wvT, "cb": cb, "tb": tb})
    return in_maps


def _gather(results, bv):
    bv = np.asarray(bv, np.float32)
    out = np.empty((B, S, H, DH), np.float32)
    for c in range(NCORES):
        o = np.asarray(results[c]["out"])  # [HPC, 65, S]
        for i in range(HPC):
            f = c * HPC + i
            b, h = f // H, f % H
            out[b, :, h, :] = (o[i, 0:64] / o[i, 64:65]).T + bv[h][None, :]
    return out.reshape(B, S, D)


def kernel(sequences, wq, bq, wk, bk, wv, bv):
    from concourse.bass_utils import run_bass_kernel_spmd

    nc = build()
    in_maps = _core_inputs(sequences, wq, bq, wk, bk, wv, bv)
    res = run_bass_kernel_spmd(nc, in_maps, list(range(NCORES)))
    return _gather(res.results, bv)


# revision 41
# speedup vs baseline: 1.2121x; 1.2121x over previous
"""Multi-head attention (B=4, S=2048, D=768, H=12) on 8 Trainium2 cores.

Sharding: the 48 (batch, head) pairs are data-parallel; each core gets 6.

Math restructure (exact):
  scores = (XWq^T+bq)(XWk^T+bk)^T -> softmax-invariant terms dropped:
    s_ij = x_i Wqk x_j^T + c_j   with Wqk = Wq^T Wk, c = X (bq Wk)^T
  The per-k-column bias c_j rides into the exp for free (per-partition
  bias port on ACT, add-constant on DVE), so K^T is the raw input and
  only one projection (qhat = X Wqk) is computed on device.
  V bias folds into a rank-1 (K=1) accumulating matmul.
  Normalization happens on the HOST: the kernel ships numerator rows
  plus a denominator row ([65, S] per head) produced by an appended
  ones-column in V_aug.

Engine split (the softmax exp is the throughput wall):
  per score pair, tile A -> ACT native exp (PSUM->SBUF bf16),
  tile B -> DVE Schraudolph: i16 = s*(A/8) + (B + A*c/8), bitcast bf16.
  (max|s/8| ~ 11, so i16 stays in [14k, 19k]: no overflow, no sign flip.)

Precision: fp16 x/qhat for the score matmuls (cuts bf16 rounding 8x),
bf16 P and V_aug, fp32 PSUM everywhere; host-measured rel err ~9.6e-3
at a 50/50 ACT/DVE split (gate is 2e-2).

PSUM: mm pool 3x[128,1024]f32 (score pairs / qhat / V) + av pool
2x[65,512]f32 = exactly 8 banks.
"""

import sys
from collections import deque

for _p in ("/opt/trn_rl_repo",):
    if _p not in sys.path:
        sys.path.insert(0, _p)

import numpy as np

B, S, D, H = 4, 2048, 768, 12
DH = 64
NCORES = 8
HPC = (B * H) // NCORES  # 6 heads per core
NKC = S // 128  # 16 k-chunks
NQB = 2  # q blocks of 1024
QB = S // NQB
PUMPS_PER_PAIR = 2

TRICK_A = 128.0 / np.log(2.0)  # 184.664965
TRICK_B = 127.0 * 128.0 - 5.57  # bf16 exponent bias, sawtooth-centered


def _split_multi_waits(nc):
    """This walrus build rejects >1 sync wait per instruction. Insert
    single-wait NoOps (same engine, so same instruction stream) ahead of
    any instruction carrying several waits."""
    import bass_rust
    import concourse.mybir as mybir

    n_split = 0
    for f in nc.m.functions:
        for bb in f.blocks:
            out = []
            dirty = False
            for inst in bb.instructions:
                si = inst.sync_info
                if si is not None and len(si.on_wait) > 1:
                    waits = list(si.on_wait)
                    for j, w in enumerate(waits[:-1]):
                        nop = mybir.InstNoOp(name=f"{inst.name}-w{j}", ins=[], outs=[])
                        nop.engine = inst.engine
                        nop.sync_info = bass_rust.SyncInfo(on_wait=[w], on_update=[])
                        out.append(nop)
                    si.on_wait = waits[-1:]
                    dirty = True
                    n_split += 1
                out.append(inst)
            if dirty:
                bb.instructions = out
    return n_split


_BUILT = None


def build():
    global _BUILT
    if _BUILT is not None:
        return _BUILT
    import concourse.bass as bass
    import concourse.mybir as mybir
    import concourse.tile as tile

    F32 = mybir.dt.float32
    F16 = mybir.dt.float16
    BF = mybir.dt.bfloat16
    I16 = mybir.dt.int16
    AF = mybir.ActivationFunctionType
    ALU = mybir.AluOpType

    nc = bass.Bass()
    xTd = nc.dram_tensor("xT", [HPC, 128, S], F16, kind="ExternalInput")
    wqkd = nc.dram_tensor("wqk", [HPC, 64, 64], F16, kind="ExternalInput")
    wvTd = nc.dram_tensor("wvT", [HPC, 128, 64], F16, kind="ExternalInput")
    cbd = nc.dram_tensor("cb", [HPC, 128, NKC], F32, kind="ExternalInput")
    tbd = nc.dram_tensor("tb", [HPC, 128, NKC], F32, kind="ExternalInput")
    outd = nc.dram_tensor("out", [HPC, 65, S], F32, kind="ExternalOutput")

    with tile.TileContext(nc) as tc:
        with (
            tc.tile_pool(name="x", bufs=2) as xpool,
            tc.tile_pool(name="w", bufs=2) as wpool,
            tc.tile_pool(name="qh", bufs=2) as qpool,
            tc.tile_pool(name="v", bufs=2) as vpool,
            tc.tile_pool(name="pt", bufs=2 * NKC) as ptpool,
            tc.tile_pool(name="ot", bufs=2) as otpool,
            tc.tile_pool(name="mm", bufs=3, space="PSUM") as mmpool,
            tc.tile_pool(name="av", bufs=2, space="PSUM") as avpool,
        ):
            # warm the ACT exp table during the first DMAs
            warm = xpool.tile([1, 1], F32, tag="warm")
            nc.vector.memset(warm[:], 0.0)
            nc.scalar.activation(warm[:], warm[:], AF.Exp)

            state = {}

            def dma_steps(i):
                x_t = xpool.tile([128, S], F16, tag="x", name=f"x{i}")
                nc.gpsimd.dma_start(x_t[:], xTd[i])
                wqk_t = wpool.tile([64, 64], F16, tag="wqk", name=f"wqk{i}")
                nc.sync.dma_start(wqk_t[:], wqkd[i])
                wv_t = wpool.tile([128, 64], F16, tag="wv", name=f"wv{i}")
                nc.sync.dma_start(wv_t[:], wvTd[i])
                cb_t = wpool.tile([128, NKC], F32, tag="cb", name=f"cb{i}")
                nc.sync.dma_start(cb_t[:], cbd[i])
                tb_t = wpool.tile([128, NKC], F32, tag="tb", name=f"tb{i}")
                nc.sync.dma_start(tb_t[:], tbd[i])
                state.setdefault(i, {})["in"] = (x_t, wqk_t, wv_t, cb_t, tb_t)
                yield

            def qkv_steps(i):
                x_t, wqk_t, wv_t, cb_t, tb_t = state[i]["in"]

                # qhat^T = Wqk^T X^T, duplicated into both partition halves
                qhat = qpool.tile([128, S], F16, tag="qh", name=f"qh{i}")
                for qm in range(NQB):
                    ps = mmpool.tile([128, QB], F32, tag="mm", name=f"qp{i}_{qm}")
                    sl = slice(qm * QB, (qm + 1) * QB)
                    for hh in range(2):
                        q0 = qm * QB + hh * 512
                        rhs = x_t[0:64, q0 : q0 + 512]
                        psl = slice(hh * 512, (hh + 1) * 512)
                        nc.tensor.matmul(
                            ps[0:64, psl], wqk_t[:], rhs, tile_position=(0, 0)
                        )
                        nc.tensor.matmul(
                            ps[64:128, psl], wqk_t[:], rhs, tile_position=(0, 64)
                        )
                    nc.scalar.activation(qhat[:, sl], ps[:], AF.Copy)
                    yield

                # V_aug: [k-in-chunk, chunk, e + ones]; V bias added on host
                vhat = vpool.tile([128, NKC, 65], BF, tag="v", name=f"v{i}")
                nc.vector.memset(vhat[:, :, 64:65], 1.0)
                vps = mmpool.tile([128, NKC * 64], F32, tag="mm", name=f"vp{i}")
                for p in range(8):
                    nc.tensor.matmul(
                        vps[:, p * 64 : (p + 1) * 64],
                        x_t[0:64, p * 128 : (p + 1) * 128],
                        wv_t[0:64, :],
                        tile_position=(0, 0),
                    )
                    nc.tensor.matmul(
                        vps[:, (p + 8) * 64 : (p + 9) * 64],
                        x_t[64:128, (p + 8) * 128 : (p + 9) * 128],
                        wv_t[64:128, :],
                        tile_position=(64, 0),
                    )
                    if p % 2 == 1:
                        yield
                nc.vector.tensor_copy(
                    vhat[:, :, 0:64],
                    vps[:].rearrange("p (c e) -> p c e", e=64),
                )
                yield
                state[i].update(
                    {"x": x_t, "qh": qhat, "v": vhat, "cb": cb_t, "tb": tb_t, "pt": {}}
                )

            def sc_pair(i, jb, p):
                """k-chunk pair (p, p+8) scores + exps for q block jb.
                Chunk p -> ACT native exp; chunk p+8 -> DVE bit-trick."""
                st = state[i]
                x_t, qhat = st["x"], st["qh"]
                pt = st["pt"].setdefault(jb, [None] * NKC)
                qsl = slice(jb * QB, (jb + 1) * QB)
                # interleave A/B halves: starts are pc-monotone, so issuing
                # A0,A1,B0,B1 would stall B0 behind A1's row-group wait
                tA = mmpool.tile([128, QB], F32, tag="mm", name=f"sA{i}_{jb}_{p}")
                tB = mmpool.tile([128, QB], F32, tag="mm", name=f"sB{i}_{jb}_{p}")
                lA = x_t[0:64, p * 128 : (p + 1) * 128]
                lB = x_t[64:128, (p + 8) * 128 : (p + 9) * 128]
                for hh in range(2):
                    q0 = jb * QB + hh * 512
                    sl = slice(hh * 512, (hh + 1) * 512)
                    nc.tensor.matmul(
                        tA[:, sl], lA, qhat[0:64, q0 : q0 + 512], tile_position=(0, 0)
                    )
                    nc.tensor.matmul(
                        tB[:, sl],
                        lB,
                        qhat[64:128, q0 : q0 + 512],
                        tile_position=(64, 0),
                    )
                pA = ptpool.tile([128, QB], BF, tag="pt", name=f"pA{i}_{jb}_{p}")
                nc.scalar.activation(
                    pA[:], tA[:], AF.Exp, bias=st["cb"][:, p : p + 1], scale=0.125
                )
                pB = ptpool.tile([128, QB], I16, tag="pt", name=f"pB{i}_{jb}_{p}")
                nc.vector.tensor_scalar(
                    pB[:],
                    tB[:],
                    TRICK_A / 8.0,
                    st["tb"][:, p + 8 : p + 9],
                    ALU.mult,
                    ALU.add,
                )
                pt[p + 8] = pB[:].bitcast(BF)
                pt[p] = pA[:]

            def av_steps(i, jb):
                """AV numerator + denominator row. The K=128 contraction is
                split into two row-group chains (k-rows 0:64 -> bank A at
                tile (0,0), rows 64:128 -> bank B at (64,0)): alternating
                half-row matmuls let each LDWEIGHTS overlap the other
                chain's matmul. The A+B merge is a DVE add that doubles as
                the PSUM->SBUF move. Output leaves unnormalized ([65, S]
                fp32); the host divides."""
                st = state[i]
                vhat = st["v"]
                pt = st["pt"][jb]
                nq = QB // 512
                avs = [
                    avpool.tile([65, 512], F32, tag="av", name=f"av{i}_{jb}_{qm}")
                    for qm in range(nq)
                ]
                for kc in range(NKC):
                    for qm in range(nq):
                        nc.tensor.matmul(
                            avs[qm][:],
                            vhat[:, kc, :],
                            pt[kc][:, qm * 512 : (qm + 1) * 512],
                            start=(kc == 0),
                            stop=(kc == NKC - 1),
                        )
                    yield
                del st["pt"][jb]
                for qm in range(nq):
                    ots = otpool.tile(
                        [65, 512], F32, tag="ot", name=f"ot{i}_{jb}_{qm}"
                    )
                    nc.scalar.activation(ots[:], avs[qm][:], AF.Copy)
                    nc.sync.dma_start(
                        outd[i][:, jb * QB + qm * 512 : jb * QB + (qm + 1) * 512],
                        ots[:],
                    )
                    yield

            fillers = deque()

            def pump(n):
                while n > 0 and fillers:
                    try:
                        next(fillers[0])
                        n -= 1
                    except StopIteration:
                        fillers.popleft()

            def drain(gen=None):
                while fillers and (gen is None or gen in fillers):
                    pump(1)

            def unit(i, jb):
                for p in range(NKC // 2):
                    sc_pair(i, jb, p)
                    pump(PUMPS_PER_PAIR)

            for _ in dma_steps(0):
                pass
            g0 = qkv_steps(0)
            fillers.append(g0)
            drain(g0)
            if HPC > 1:
                fillers.append(dma_steps(1))
            unit(0, 0)
            for i in range(HPC):
                if i > 0:
                    if i + 1 < HPC:
                        fillers.append(dma_steps(i + 1))
                    fillers.append(av_steps(i - 1, 1))
                    unit(i, 0)
                fillers.append(av_steps(i, 0))
                if i + 1 < HPC:
                    g = qkv_steps(i + 1)
                    fillers.append(g)
                    unit(i, 1)
                    drain(g)
                else:
                    unit(i, 1)
            fillers.append(av_steps(HPC - 1, 1))
            drain()

    _split_multi_waits(nc)
    _BUILT = nc
    return nc


def _core_inputs(sequences, wq, bq, wk, bk, wv, bv):
    f16 = np.float16
    xh = np.asarray(sequences, dtype=np.float32).reshape(B, S, H, DH)
    wq = np.asarray(wq, np.float32)
    bq = np.asarray(bq, np.float32)
    wk = np.asarray(wk, np.float32)
    wv = np.asarray(wv, np.float32)
    bv = np.asarray(bv, np.float32)
    in_maps = []
    for c in range(NCORES):
        xT = np.empty((HPC, 128, S), dtype=f16)
        wqk = np.empty((HPC, 64, 64), dtype=f16)
        wvT = np.empty((HPC, 128, 64), dtype=f16)
        cb = np.empty((HPC, 128, NKC), dtype=np.float32)
        tb = np.empty((HPC, 128, NKC), dtype=np.float32)
        for i in range(HPC):
            f = c * HPC + i
            b, h = f // H, f % H
            xbh = xh[b, :, h, :]  # [S, 64]
            xt = np.ascontiguousarray(xbh.T).astype(f16)
            xT[i, 0:64] = xt
            xT[i, 64:128] = xt
            wqk[i] = (wq[h].T @ wk[h]).astype(f16)
            wvT[i, 0:64] = wv[h].T.astype(f16)
            wvT[i, 64:128] = wv[h].T.astype(f16)
            btil = bq[h] @ wk[h]  # [64]
            c8 = (xbh @ btil) / 8.0  # [S]
            cb[i] = c8.reshape(NKC, 128).T
            tb[i] = TRICK_B + TRICK_A * cb[i]
        in_maps.append({"xT": xT, "wqk": wqk, "wvT": wvT, "cb": cb, "tb": tb})
    return in_maps


def _gather(results, bv):
    bv = np.asarray(bv, np.float32)
    out = np.empty((B, S, H, DH), np.float32)
    for c in range(NCORES):
        o = np.asarray(results[c]["out"])  # [HPC, 65, S]
        for i in range(HPC):
            f = c * HPC + i
            b, h = f // H, f % H
            out[b, :, h, :] = (o[i, 0:64] / o[i, 64:65]).T + bv[h][None, :]
    return out.reshape(B, S, D)


def kernel(sequences, wq, bq, wk, bk, wv, bv):
    from concourse.bass_utils import run_bass_kernel_spmd

    nc = build()
    in_maps = _core_inputs(sequences, wq, bq, wk, bk, wv, bv)
    res = run_bass_kernel_spmd(nc, in_maps, list(range(NCORES)))
    return _gather(res.results, bv)


# revision 43
# speedup vs baseline: 1.2359x; 1.0196x over previous
"""Multi-head attention (B=4, S=2048, D=768, H=12) on 8 Trainium2 cores.

Sharding: the 48 (batch, head) pairs are data-parallel; each core gets 6.

Math restructure (exact):
  scores = (XWq^T+bq)(XWk^T+bk)^T -> softmax-invariant terms dropped:
    s_ij = x_i Wqk x_j^T + c_j   with Wqk = Wq^T Wk, c = X (bq Wk)^T
  The per-k-column bias c_j rides into the exp for free (per-partition
  bias port on ACT, add-constant on DVE), so K^T is the raw input and
  only one projection (qhat = X Wqk) is computed on device.
  V bias folds into a rank-1 (K=1) accumulating matmul.
  Normalization happens on the HOST: the kernel ships numerator rows
  plus a denominator row ([65, S] per head) produced by an appended
  ones-column in V_aug.

Engine split (the softmax exp is the throughput wall):
  per score pair, tile A -> ACT native exp (PSUM->SBUF bf16),
  tile B -> DVE Schraudolph: i16 = s*(A/8) + (B + A*c/8), bitcast bf16.
  (max|s/8| ~ 11, so i16 stays in [14k, 19k]: no overflow, no sign flip.)

Precision: fp16 x/qhat for the score matmuls (cuts bf16 rounding 8x),
bf16 P and V_aug, fp32 PSUM everywhere; host-measured rel err ~9.6e-3
at a 50/50 ACT/DVE split (gate is 2e-2).

PSUM: mm pool 3x[128,1024]f32 (score pairs / qhat / V) + av pool
2x[65,512]f32 = exactly 8 banks.
"""

import sys
from collections import deque

for _p in ("/opt/trn_rl_repo",):
    if _p not in sys.path:
        sys.path.insert(0, _p)

import numpy as np

B, S, D, H = 4, 2048, 768, 12
DH = 64
NCORES = 8
HPC = (B * H) // NCORES  # 6 heads per core
NKC = S // 128  # 16 k-chunks
NQB = 2  # q blocks of 1024
QB = S // NQB
PUMPS_PER_PAIR = 2

TRICK_A = 128.0 / np.log(2.0)  # 184.664965
TRICK_B = 127.0 * 128.0 - 5.57  # bf16 exponent bias, sawtooth-centered


def _split_multi_waits(nc):
    """This walrus build rejects >1 sync wait per instruction. Insert
    single-wait NoOps (same engine, so same instruction stream) ahead of
    any instruction carrying several waits."""
    import bass_rust
    import concourse.mybir as mybir

    n_split = 0
    for f in nc.m.functions:
        for bb in f.blocks:
            out = []
            dirty = False
            for inst in bb.instructions:
                si = inst.sync_info
                if si is not None and len(si.on_wait) > 1:
                    waits = list(si.on_wait)
                    for j, w in enumerate(waits[:-1]):
                        nop = mybir.InstNoOp(name=f"{inst.name}-w{j}", ins=[], outs=[])
                        nop.engine = inst.engine
                        nop.sync_info = bass_rust.SyncInfo(on_wait=[w], on_update=[])
                        out.append(nop)
                    si.on_wait = waits[-1:]
                    dirty = True
                    n_split += 1
                out.append(inst)
            if dirty:
                bb.instructions = out
    return n_split


_BUILT = None


def build():
    global _BUILT
    if _BUILT is not None:
        return _BUILT
    import concourse.bass as bass
    import concourse.mybir as mybir
    import concourse.tile as tile

    F32 = mybir.dt.float32
    F16 = mybir.dt.float16
    BF = mybir.dt.bfloat16
    I16 = mybir.dt.int16
    AF = mybir.ActivationFunctionType
    ALU = mybir.AluOpType

    nc = bass.Bass()
    xTd = nc.dram_tensor("xT", [HPC, 128, S], F16, kind="ExternalInput")
    wqkd = nc.dram_tensor("wqk", [HPC, 64, 64], F16, kind="ExternalInput")
    wvTd = nc.dram_tensor("wvT", [HPC, 128, 64], F16, kind="ExternalInput")
    cbd = nc.dram_tensor("cb", [HPC, 128, NKC], F32, kind="ExternalInput")
    tbd = nc.dram_tensor("tb", [HPC, 128, NKC], F32, kind="ExternalInput")
    outd = nc.dram_tensor("out", [HPC, 65, S], F32, kind="ExternalOutput")

    with tile.TileContext(nc) as tc:
        with (
            tc.tile_pool(name="x", bufs=2) as xpool,
            tc.tile_pool(name="w", bufs=2) as wpool,
            tc.tile_pool(name="qh", bufs=2) as qpool,
            tc.tile_pool(name="v", bufs=2) as vpool,
            tc.tile_pool(name="pt", bufs=2 * NKC) as ptpool,
            tc.tile_pool(name="ot", bufs=2) as otpool,
            tc.tile_pool(name="mm", bufs=3, space="PSUM") as mmpool,
            tc.tile_pool(name="av", bufs=2, space="PSUM") as avpool,
        ):
            # warm the ACT exp table during the first DMAs
            warm = xpool.tile([1, 1], F32, tag="warm")
            nc.vector.memset(warm[:], 0.0)
            nc.scalar.activation(warm[:], warm[:], AF.Exp)

            state = {}

            def qkv_steps(i):
                x_t = xpool.tile([128, S], F16, tag="x", name=f"x{i}")
                nc.gpsimd.dma_start(x_t[:], xTd[i])
                wqk_t = wpool.tile([64, 64], F16, tag="wqk", name=f"wqk{i}")
                nc.sync.dma_start(wqk_t[:], wqkd[i])
                wv_t = wpool.tile([128, 64], F16, tag="wv", name=f"wv{i}")
                nc.sync.dma_start(wv_t[:], wvTd[i])
                cb_t = wpool.tile([128, NKC], F32, tag="cb", name=f"cb{i}")
                nc.sync.dma_start(cb_t[:], cbd[i])
                tb_t = wpool.tile([128, NKC], F32, tag="tb", name=f"tb{i}")
                nc.sync.dma_start(tb_t[:], tbd[i])
                state.setdefault(i, {})
                yield

                # qhat^T = Wqk^T X^T, duplicated into both partition halves
                qhat = qpool.tile([128, S], F16, tag="qh", name=f"qh{i}")
                for qm in range(NQB):
                    ps = mmpool.tile([128, QB], F32, tag="mm", name=f"qp{i}_{qm}")
                    sl = slice(qm * QB, (qm + 1) * QB)
                    for hh in range(2):
                        q0 = qm * QB + hh * 512
                        rhs = x_t[0:64, q0 : q0 + 512]
                        psl = slice(hh * 512, (hh + 1) * 512)
                        nc.tensor.matmul(
                            ps[0:64, psl], wqk_t[:], rhs, tile_position=(0, 0)
                        )
                        nc.tensor.matmul(
                            ps[64:128, psl], wqk_t[:], rhs, tile_position=(0, 64)
                        )
                    nc.scalar.activation(qhat[:, sl], ps[:], AF.Copy)
                    yield

                # V_aug: [k-in-chunk, chunk, e + ones]; V bias added on host
                vhat = vpool.tile([128, NKC, 65], BF, tag="v", name=f"v{i}")
                nc.vector.memset(vhat[:, :, 64:65], 1.0)
                vps = mmpool.tile([128, NKC * 64], F32, tag="mm", name=f"vp{i}")
                for p in range(8):
                    nc.tensor.matmul(
                        vps[:, p * 64 : (p + 1) * 64],
                        x_t[0:64, p * 128 : (p + 1) * 128],
                        wv_t[0:64, :],
                        tile_position=(0, 0),
                    )
                    nc.tensor.matmul(
                        vps[:, (p + 8) * 64 : (p + 9) * 64],
                        x_t[64:128, (p + 8) * 128 : (p + 9) * 128],
                        wv_t[64:128, :],
                        tile_position=(64, 0),
                    )
                    if p % 2 == 1:
                        yield
                nc.vector.tensor_copy(
                    vhat[:, :, 0:64],
                    vps[:].rearrange("p (c e) -> p c e", e=64),
                )
                yield
                state[i].update(
                    {"x": x_t, "qh": qhat, "v": vhat, "cb": cb_t, "tb": tb_t, "pt": {}}
                )

            def sc_pair(i, jb, p):
                """k-chunk pair (p, p+8) scores + exps for q block jb.
                Chunk p -> ACT native exp; chunk p+8 -> DVE bit-trick."""
                st = state[i]
                x_t, qhat = st["x"], st["qh"]
                pt = st["pt"].setdefault(jb, [None] * NKC)
                qsl = slice(jb * QB, (jb + 1) * QB)
                # interleave A/B halves: starts are pc-monotone, so issuing
                # A0,A1,B0,B1 would stall B0 behind A1's row-group wait
                tA = mmpool.tile([128, QB], F32, tag="mm", name=f"sA{i}_{jb}_{p}")
                tB = mmpool.tile([128, QB], F32, tag="mm", name=f"sB{i}_{jb}_{p}")
                lA = x_t[0:64, p * 128 : (p + 1) * 128]
                lB = x_t[64:128, (p + 8) * 128 : (p + 9) * 128]
                for hh in range(2):
                    q0 = jb * QB + hh * 512
                    sl = slice(hh * 512, (hh + 1) * 512)
                    nc.tensor.matmul(
                        tA[:, sl], lA, qhat[0:64, q0 : q0 + 512], tile_position=(0, 0)
                    )
                    nc.tensor.matmul(
                        tB[:, sl],
                        lB,
                        qhat[64:128, q0 : q0 + 512],
                        tile_position=(64, 0),
                    )
                pA = ptpool.tile([128, QB], BF, tag="pt", name=f"pA{i}_{jb}_{p}")
                nc.scalar.activation(
                    pA[:], tA[:], AF.Exp, bias=st["cb"][:, p : p + 1], scale=0.125
                )
                pB = ptpool.tile([128, QB], I16, tag="pt", name=f"pB{i}_{jb}_{p}")
                nc.vector.tensor_scalar(
                    pB[:],
                    tB[:],
                    TRICK_A / 8.0,
                    st["tb"][:, p + 8 : p + 9],
                    ALU.mult,
                    ALU.add,
                )
                pt[p + 8] = pB[:].bitcast(BF)
                pt[p] = pA[:]

            def av_steps(i, jb):
                """AV numerator + denominator row. The K=128 contraction is
                split into two row-group chains (k-rows 0:64 -> bank A at
                tile (0,0), rows 64:128 -> bank B at (64,0)): alternating
                half-row matmuls let each LDWEIGHTS overlap the other
                chain's matmul. The A+B merge is a DVE add that doubles as
                the PSUM->SBUF move. Output leaves unnormalized ([65, S]
                fp32); the host divides."""
                st = state[i]
                vhat = st["v"]
                pt = st["pt"][jb]
                nq = QB // 512
                avs = [
                    avpool.tile([65, 512], F32, tag="av", name=f"av{i}_{jb}_{qm}")
                    for qm in range(nq)
                ]
                for kc in range(NKC):
                    for qm in range(nq):
                        nc.tensor.matmul(
                            avs[qm][:],
                            vhat[:, kc, :],
                            pt[kc][:, qm * 512 : (qm + 1) * 512],
                            start=(kc == 0),
                            stop=(kc == NKC - 1),
                        )
                    yield
                del st["pt"][jb]
                for qm in range(nq):
                    ots = otpool.tile(
                        [65, 512], F32, tag="ot", name=f"ot{i}_{jb}_{qm}"
                    )
                    nc.scalar.activation(ots[:], avs[qm][:], AF.Copy)
                    nc.sync.dma_start(
                        outd[i][:, jb * QB + qm * 512 : jb * QB + (qm + 1) * 512],
                        ots[:],
                    )
                    yield

            fillers = deque()

            def pump(n):
                while n > 0 and fillers:
                    try:
                        next(fillers[0])
                        n -= 1
                    except StopIteration:
                        fillers.popleft()

            def drain(gen=None):
                while fillers and (gen is None or gen in fillers):
                    pump(1)

            def unit(i, jb):
                for p in range(NKC // 2):
                    sc_pair(i, jb, p)
                    pump(PUMPS_PER_PAIR)

            g0 = qkv_steps(0)
            fillers.append(g0)
            drain(g0)
            unit(0, 0)
            for i in range(HPC):
                if i > 0:
                    fillers.append(av_steps(i - 1, 1))
                    unit(i, 0)
                fillers.append(av_steps(i, 0))
                if i + 1 < HPC:
                    g = qkv_steps(i + 1)
                    fillers.append(g)
                    unit(i, 1)
                    drain(g)
                else:
                    unit(i, 1)
            fillers.append(av_steps(HPC - 1, 1))
            drain()

    _split_multi_waits(nc)
    _BUILT = nc
    return nc


def _core_inputs(sequences, wq, bq, wk, bk, wv, bv):
    f16 = np.float16
    xh = np.asarray(sequences, dtype=np.float32).reshape(B, S, H, DH)
    wq = np.asarray(wq, np.float32)
    bq = np.asarray(bq, np.float32)
    wk = np.asarray(wk, np.float32)
    wv = np.asarray(wv, np.float32)
    bv = np.asarray(bv, np.float32)
    in_maps = []
    for c in range(NCORES):
        xT = np.empty((HPC, 128, S), dtype=f16)
        wqk = np.empty((HPC, 64, 64), dtype=f16)
        wvT = np.empty((HPC, 128, 64), dtype=f16)
        cb = np.empty((HPC, 128, NKC), dtype=np.float32)
        tb = np.empty((HPC, 128, NKC), dtype=np.float32)
        for i in range(HPC):
            f = c * HPC + i
            b, h = f // H, f % H
            xbh = xh[b, :, h, :]  # [S, 64]
            xt = np.ascontiguousarray(xbh.T).astype(f16)
            xT[i, 0:64] = xt
            xT[i, 64:128] = xt
            wqk[i] = (wq[h].T @ wk[h]).astype(f16)
            wvT[i, 0:64] = wv[h].T.astype(f16)
            wvT[i, 64:128] = wv[h].T.astype(f16)
            btil = bq[h] @ wk[h]  # [64]
            c8 = (xbh @ btil) / 8.0  # [S]
            cb[i] = c8.reshape(NKC, 128).T
            tb[i] = TRICK_B + TRICK_A * cb[i]
        in_maps.append({"xT": xT, "wqk": wqk, "wvT": wvT, "cb": cb, "tb": tb})
    return in_maps


def _gather(results, bv):
    bv = np.asarray(bv, np.float32)
    out = np.empty((B, S, H, DH), np.float32)
    for c in range(NCORES):
        o = np.asarray(results[c]["out"])  # [HPC, 65, S]
        for i in range(HPC):
            f = c * HPC + i
            b, h = f // H, f % H
            out[b, :, h, :] = (o[i, 0:64] / o[i, 64:65]).T + bv[h][None, :]
    return out.reshape(B, S, D)


def kernel(sequences, wq, bq, wk, bk, wv, bv):
    from concourse.bass_utils import run_bass_kernel_spmd

    nc = build()
    in_maps = _core_inputs(sequences, wq, bq, wk, bk, wv, bv)
    res = run_bass_kernel_spmd(nc, in_maps, list(range(NCORES)))
    return _gather(res.results, bv)
